# revision 26
# baseline (speedup 1.0000x reference)
"""Causal self-attention (B=2, T=2048, C=2048, H=16) on 8 TRN2 NeuronCores.

Sharding: tensor-parallel over heads (2 heads per core, both batches on every
core). Each core computes q/k/v projections for its 2 heads, RoPE, causal
softmax(qk^T)v, and a partial output projection against its slice of Wo's
columns. The host sums the 8 partial projections and adds the (linear) bias
terms (bo + Wo @ bv).

v2 changes vs the f32r baseline (615us -> target ~300us):
  - All matmul operands fp16 (PSUM accumulation stays f32). The correctness
    budget is huge (baseline rel err 3.4e-4 vs the 2e-2 gate); fp16 halves
    DMA traffic, doubles DVE throughput, and halves LDWEIGHTS time.
  - Causal mask folded into the score accumulation: a [128,W] matmul writes
    -60000 into the masked triangle (identity lhsT x const mask rhs,
    start=True) and the k.q matmul accumulates on top. exp then underflows
    to exactly 0. This removes the DVE mask multiply from the
    scores->exp->PV critical path.
  - Global emission scheduler: the PE executes its queue in order, so the
    Python emission order IS the schedule. Attention streams (latency-bound:
    s -> exp on ACT -> y) are round-robined two at a time, and every slot is
    padded with "bulk" matmuls (qkv projections of the *next* batch, output
    projections of finished blocks). The PE therefore never idles, which
    also keeps it at the 2.4 GHz p-state (it drops to 1.2 GHz whenever the
    pipeline has gaps - that is where the baseline lost most of its time).
  - Output projection tiles are DMA'd straight out of PSUM (f32) on the
    sync queue; no PSUM->SBUF copy.

Layout (unchanged): x pre-transposed on host to xT[b] [C,T]; q,k produced
transposed [head_dim, T]; scores S_T = k_tile.T @ qT [keys, queries] so the
softmax reduction runs along partitions via a ones-matmul; y produced
transposed yT = v_tile.T @ P_T; output projection out = yT_slice.T @ wo.
"""

import sys

sys.path.insert(0, "/opt/trn_rl_repo")

from collections import deque

import numpy as np

import concourse.bacc as bacc
import concourse.mybir as mybir
import concourse.tile as tile
from concourse import bass_utils

B, T, C, H = 2, 2048, 2048, 16
HD = C // H  # 128
BASE = 10000.0
NC_ = 8  # cores
NH = H // NC_  # heads per core = 2
TB = 512  # T block
NTB = T // TB  # 4
CK = C // 128  # 16 contraction chunks
SCALE = 1.0 / float(np.sqrt(np.float32(HD)))
MASKVAL = -60000.0

f32 = mybir.dt.float32
f16 = mybir.dt.float16
AF = mybir.ActivationFunctionType
OP = mybir.AluOpType

TRACE = False
LAST_RESULT = None
LDW_OPT = False

_orig_run_command = bass_utils.run_command


def _patched_run_command(cmd, **kw):
    if LDW_OPT and isinstance(cmd, list):
        cmd = [
            ("--enable-ldw-opt=true" if c == "--enable-ldw-opt=false" else c)
            for c in cmd
        ]
    return _orig_run_command(cmd, **kw)


bass_utils.run_command = _patched_run_command

_STATE = {}


def _rope_tables():
    """cos/sin tables [HD, T] mirroring reference._rope_tables (f32 chain)."""
    try:
        import jax
        import jax.numpy as jnp

        cpu = jax.devices("cpu")[0]
        with jax.default_device(cpu):
            p = jnp.arange(HD // 2, dtype=jnp.float32)
            theta = jnp.power(BASE, -(2.0**p) / HD)
            pos = jnp.arange(1, T + 1, dtype=jnp.float32)[:, None]
            c = pos * theta
            ang = jnp.concatenate([c, c], axis=-1)  # [T, HD]
            cos = np.asarray(jnp.cos(ang)).T  # [HD, T]
            sin = np.asarray(jnp.sin(ang)).T
        return np.ascontiguousarray(cos), np.ascontiguousarray(sin)
    except Exception:
        p = np.arange(HD // 2, dtype=np.float32)
        theta = np.power(np.float32(BASE), (-(2.0**p) / HD).astype(np.float32))
        pos = np.arange(1, T + 1, dtype=np.float32)[:, None]
        c = (pos * theta).astype(np.float32)
        ang = np.concatenate([c, c], axis=-1)
        return (
            np.ascontiguousarray(np.cos(ang).T.astype(np.float32)),
            np.ascontiguousarray(np.sin(ang).T.astype(np.float32)),
        )


def _build_program():
    nc = bacc.Bacc("TRN2", target_bir_lowering=False, debug=False, num_devices=NC_)

    d_xT = nc.dram_tensor("xT", (B, C, T), f16, kind="ExternalInput")
    d_wq = nc.dram_tensor("wq", (C, NH * HD), f16, kind="ExternalInput")
    d_wk = nc.dram_tensor("wk", (C, NH * HD), f16, kind="ExternalInput")
    d_wv = nc.dram_tensor("wv", (C, NH * HD), f16, kind="ExternalInput")
    d_wo = nc.dram_tensor("wo", (NH * HD, C), f16, kind="ExternalInput")
    d_bq = nc.dram_tensor("bq", (HD, NH), f32, kind="ExternalInput")
    d_bk = nc.dram_tensor("bk", (HD, NH), f32, kind="ExternalInput")
    d_cos = nc.dram_tensor("cosT", (HD, T), f16, kind="ExternalInput")
    d_sin = nc.dram_tensor("sinT", (HD, T), f16, kind="ExternalInput")
    d_maskM = nc.dram_tensor("maskM", (128, TB), f16, kind="ExternalInput")
    d_id = nc.dram_tensor("ident", (128, 128), f16, kind="ExternalInput")
    d_ones = nc.dram_tensor("onesm", (128, 128), f16, kind="ExternalInput")
    d_out = nc.dram_tensor("out", (B, T, C), f16, kind="ExternalOutput")

    with tile.TileContext(nc) as tc:
        with (
            tc.tile_pool(name="w", bufs=1) as wp,
            tc.tile_pool(name="xp", bufs=1) as xp,
            tc.tile_pool(name="kv", bufs=1) as kvp,
            tc.tile_pool(name="wk", bufs=1) as wkp,
            tc.tile_pool(name="ps", bufs=1, space="PSUM") as ps,
        ):
            # --- resident weights/constants ---
            # tiny consts first so PE warmup can start within ~0.5us
            maskM_t = wp.tile([128, TB], f16, name="maskM_t")
            nc.gpsimd.dma_start(maskM_t[:], d_maskM.ap()[:])
            id_t = wp.tile([128, 128], f16, name="id_t")
            nc.gpsimd.dma_start(id_t[:], d_id.ap()[:])
            ones_t = wp.tile([128, 128], f16, name="ones_t")
            nc.gpsimd.dma_start(ones_t[:], d_ones.ap()[:])
            # PE warmup: matmuls on the const tiles ramp the tensor-engine
            # p-state while the initial weight/x DMAs are still in flight.
            warm_ps = ps.tile([128, TB], f32, tag="fill", bufs=3, name="warm_ps")
            for _ in range(20):
                nc.tensor.matmul(
                    warm_ps[:], ones_t[:], maskM_t[:], skip_group_check=True
                )
            wq_t = wp.tile([128, CK, NH * HD], f16, name="wq_t")
            wk_t = wp.tile([128, CK, NH * HD], f16, name="wk_t")
            wv_t = wp.tile([128, CK, NH * HD], f16, name="wv_t")
            wq_src = d_wq.ap().rearrange("(k p) n -> p k n", p=128)
            wk_src = d_wk.ap().rearrange("(k p) n -> p k n", p=128)
            bq_t = wp.tile([128, NH], f32, name="bq_t")
            nc.gpsimd.dma_start(bq_t[:], d_bq.ap()[:])
            bk_t = wp.tile([128, NH], f32, name="bk_t")
            nc.gpsimd.dma_start(bk_t[:], d_bk.ap()[:])
            # wq on the scalar queue; wk split across scalar+gpsimd so the
            # last quads land before the k-projection groups reach them
            for qd in range(4):
                qs = slice(qd * 4, (qd + 1) * 4)
                nc.scalar.dma_start(wq_t[:, qs, :], wq_src[:, qs, :])
                if qd < 2:
                    nc.scalar.dma_start(wk_t[:, qs, :], wk_src[:, qs, :])
                else:
                    nc.gpsimd.dma_start(wk_t[:, qs, :], wk_src[:, qs, :])
            cos_t = wp.tile([128, T], f16, name="cos_t")
            nc.gpsimd.dma_start(cos_t[:], d_cos.ap()[:])
            sin_t = wp.tile([128, T], f16, name="sin_t")
            nc.gpsimd.dma_start(sin_t[:], d_sin.ap()[:])
            # deferred weights (needed later than wq/wk)
            wv_src = d_wv.ap().rearrange("(k p) n -> p k n", p=128)
            for qd in range(4):
                qs = slice(qd * 4, (qd + 1) * 4)
                nc.gpsimd.dma_start(wv_t[:, qs, :], wv_src[:, qs, :])
            wo_t = wp.tile([128, NH, C], f16, name="wo_t")
            for h in range(NH):
                nc.gpsimd.dma_start(
                    wo_t[:, h, :], d_wo.ap()[h * 128 : (h + 1) * 128, :]
                )

            # --- python-side bookkeeping ---
            xchunks = {}
            qTt = {}
            kTt = {}
            vtt = {}
            yTs = {}
            p1_done = {0: 0, 1: 0}
            done_h = {}
            bulk = deque()
            counters = {"attn": 0, "bulk": 0}

            def xload(b, tb):
                tiles = []
                src = d_xT.ap()[b].rearrange("(k p) t -> p k t", p=128)
                for qd in range(4):
                    xt = xp.tile(
                        [128, 4, TB], f16, tag="x", bufs=10, name=f"x{b}_{tb}_{qd}"
                    )
                    nc.sync.dma_start(
                        xt[:],
                        src[:, qd * 4 : (qd + 1) * 4, tb * TB : (tb + 1) * TB],
                    )
                    tiles.append(xt)
                xchunks[(b, tb)] = tiles

            def qk_group(b, tb, h, proj):
                w_t, b_t = (wq_t, bq_t) if proj == 0 else (wk_t, bk_t)
                tbs = slice(tb * TB, (tb + 1) * TB)
                hsl = slice(h * HD, (h + 1) * HD)
                xs = xchunks[(b, tb)]
                prj = ps.tile([128, TB], f32, tag="fill", bufs=3, name="prj")
                for kc in range(CK):
                    nc.tensor.matmul(
                        prj[:],
                        w_t[:, kc, hsl],
                        xs[kc // 4][:, kc % 4, :],
                        start=(kc == 0),
                        stop=(kc == CK - 1),
                        skip_group_check=True,
                    )
                    yield
                qb = wkp.tile([128, TB], f16, tag="qb", bufs=4, name="qb")
                nc.scalar.activation(qb[:], prj[:], AF.Identity, bias=b_t[:, h : h + 1])
                rotb = wkp.tile([128, TB], f16, tag="qb", bufs=4, name="rotb")
                nc.scalar.dma_start(rotb[0:64, :], qb[64:128, :])
                nc.scalar.dma_start(rotb[64:128, :], qb[0:64, :])
                t1 = wkp.tile([128, TB], f16, tag="rt", bufs=4, name="t1")
                nc.vector.tensor_tensor(t1[:], qb[:], cos_t[:, tbs], OP.mult)
                t2 = wkp.tile([128, TB], f16, tag="rt", bufs=4, name="t2")
                nc.vector.tensor_tensor(t2[:], rotb[:], sin_t[:, tbs], OP.mult)
                dest = (qTt if proj == 0 else kTt)[(b, h)]
                nc.vector.tensor_tensor(dest[:, tbs], t1[:], t2[:], OP.add)

            def v_group(b, tb, tt):
                xs = xchunks[(b, tb)]
                vps = ps.tile([128, TB], f32, tag="fill", bufs=3, name="vps")
                for kc in range(CK):
                    nc.tensor.matmul(
                        vps[:, : NH * HD],
                        xs[kc // 4][:, kc % 4, tt * 128 : (tt + 1) * 128],
                        wv_t[:, kc, :],
                        start=(kc == 0),
                        stop=(kc == CK - 1),
                        skip_group_check=True,
                    )
                    yield
                if tt % 2 == 0:
                    nc.scalar.activation(
                        vtt[b][:, tb * 4 + tt, :], vps[:, : NH * HD], AF.Identity
                    )
                else:
                    nc.vector.tensor_copy(vtt[b][:, tb * 4 + tt, :], vps[:, : NH * HD])

            def p1_gen(b):
                for h in range(NH):
                    qTt[(b, h)] = kvp.tile(
                        [128, T], f16, tag=f"q{h}", bufs=2, name=f"qT{b}_{h}"
                    )
                    kTt[(b, h)] = kvp.tile(
                        [128, T], f16, tag=f"k{h}", bufs=2, name=f"kT{b}_{h}"
                    )
                vtt[b] = kvp.tile(
                    [128, CK, NH * HD], f16, tag="v", bufs=2, name=f"v{b}"
                )
                for tb in range(NTB):
                    xload(b, tb)
                    for proj in range(2):
                        for h in range(NH):
                            yield from qk_group(b, tb, h, proj)
                    for tt in range(4):
                        yield from v_group(b, tb, tt)
                    p1_done[b] = tb + 1

            def out_group(b, tb, tt, ncc):
                o_ps = ps.tile([128, TB], f32, tag="fill", bufs=3, name="o_ps")
                for h in range(NH):
                    nc.tensor.matmul(
                        o_ps[:],
                        yTs[(b, tb, h)][:, tt * 128 : (tt + 1) * 128],
                        wo_t[:, h, ncc * TB : (ncc + 1) * TB],
                        start=(h == 0),
                        stop=(h == NH - 1),
                        skip_group_check=True,
                    )
                    yield
                r0 = tb * TB + tt * 128
                ot = wkp.tile([128, TB], f16, tag="ot", bufs=6, name="ot")
                if ncc % 2 == 0:
                    nc.scalar.activation(ot[:], o_ps[:], AF.Identity)
                else:
                    nc.vector.tensor_copy(ot[:], o_ps[:])
                dq = nc.gpsimd if ncc % 2 == 0 else nc.sync
                dq.dma_start(
                    d_out.ap()[b, r0 : r0 + 128, ncc * TB : (ncc + 1) * TB], ot[:]
                )

            def attn_stream(b, h, tb):
                nkt = 4 * tb + 4
                hsl = slice(h * HD, (h + 1) * HD)
                q = qTt[(b, h)]
                k = kTt[(b, h)]
                v = vtt[b]
                y_ps = ps.tile([128, TB], f32, tag="y", bufs=2, name="y_ps")
                dacc = wkp.tile([128, TB], f16, tag="dacc", bufs=4, name="dacc")
                pend = deque()
                LAG = 1

                def emit_y(item):
                    kt_, q0_, W_, pt_ = item
                    nc.tensor.matmul(
                        y_ps[:, q0_:],
                        v[:, kt_, hsl],
                        pt_[:, :W_],
                        start=(kt_ == 0),
                        stop=(kt_ == nkt - 1),
                        skip_group_check=True,
                    )

                for kt in range(nkt):
                    o = kt - 4 * tb
                    q0 = 128 * o if o > 0 else 0
                    W = TB - q0
                    s_ps = ps.tile([128, TB], f32, tag="s", bufs=3, name="s_ps")
                    if o >= 0:
                        nc.tensor.matmul(
                            s_ps[:, :W],
                            k[:, kt * 128 : (kt + 1) * 128],
                            q[:, tb * TB + q0 : (tb + 1) * TB],
                            start=True,
                            stop=False,
                            skip_group_check=True,
                        )
                        yield
                        nc.tensor.matmul(
                            s_ps[:, :128],
                            id_t[:],
                            maskM_t[:, :128],
                            start=False,
                            stop=True,
                            skip_group_check=True,
                        )
                        yield
                    else:
                        nc.tensor.matmul(
                            s_ps[:, :W],
                            k[:, kt * 128 : (kt + 1) * 128],
                            q[:, tb * TB + q0 : (tb + 1) * TB],
                            start=True,
                            stop=True,
                            skip_group_check=True,
                        )
                        yield
                    pt = wkp.tile([128, TB], f16, tag="p", bufs=9, name="pt")
                    nc.scalar.activation(pt[:, :W], s_ps[:, :W], AF.Exp, scale=SCALE)
                    if kt == 0:
                        nc.vector.tensor_copy(dacc[:], pt[:])
                    else:
                        nc.vector.tensor_tensor(
                            dacc[:, q0:], dacc[:, q0:], pt[:, :W], OP.add
                        )
                    pend.append((kt, q0, W, pt))
                    if len(pend) > LAG:
                        emit_y(pend.popleft())
                        yield
                while pend:
                    emit_y(pend.popleft())
                    yield
                den_ps = ps.tile([128, TB], f32, tag="s", bufs=3, name="den_ps")
                nc.tensor.matmul(den_ps[:], ones_t[:], dacc[:], skip_group_check=True)
                rden = wkp.tile([128, TB], f32, tag="rden", bufs=3, name="rden")
                nc.vector.reciprocal_approx_fast(rden[:], den_ps[:])
                yT = wkp.tile([128, TB], f16, tag="yT", bufs=6, name="yT")
                nc.vector.tensor_tensor(yT[:], y_ps[:], rden[:], OP.mult)
                yTs[(b, tb, h)] = yT
                done_h[(b, tb)] = done_h.get((b, tb), 0) + 1
                if done_h[(b, tb)] == NH:
                    for tt in range(4):
                        for ncc in range(4):
                            bulk.append(("p3", out_group(b, tb, tt, ncc)))
                            counters["bulk"] += 2

            # --- schedule ---
            streams = deque()
            for b in range(B):
                for tb in range(NTB):
                    for h in range(NH):
                        nkt = 4 * tb + 4
                        streams.append((b, h, tb))
                        counters["attn"] += 2 * nkt + 4

            bulk.append(("p1", p1_gen(0)))
            counters["bulk"] += 512
            bulk.append(("p1", p1_gen(1)))
            counters["bulk"] += 512

            active = []

            def refill():
                while len(active) < 2 and streams:
                    b, h, tb = streams[0]
                    if p1_done[b] >= tb + 1:
                        streams.popleft()
                        active.append(attn_stream(b, h, tb))
                    else:
                        break

            bulk_rr = []

            def bulk_step(n):
                # Step up to two bulk generators round-robin so independent
                # out-proj groups pipeline; never run two "p1" (projection)
                # generators concurrently (their x-chunk DMA order matters).
                emitted = 0
                while emitted < n:
                    while len(bulk_rr) < 2 and bulk:
                        if bulk[0][0] == "p1" and any(
                            k == "p1" for k, _ in bulk_rr
                        ):
                            break
                        bulk_rr.append(bulk.popleft())
                    if not bulk_rr:
                        return
                    item = bulk_rr.pop(0)
                    bulk_rr.append(item)
                    try:
                        next(item[1])
                        emitted += 1
                        counters["bulk"] -= 1
                    except StopIteration:
                        bulk_rr.remove(item)

            while True:
                refill()
                if not active and not bulk and not bulk_rr and not streams:
                    break
                if active:
                    for st in list(active):
                        try:
                            next(st)
                            counters["attn"] -= 1
                        except StopIteration:
                            active.remove(st)
                    r = max(
                        1,
                        min(4, round(counters["bulk"] / max(counters["attn"], 1))),
                    )
                    bulk_step(r)
                else:
                    bulk_step(4)

    nc.compile()
    return nc


def _get_program():
    if "nc" not in _STATE:
        _STATE["nc"] = _build_program()
    return _STATE["nc"]


def _enable_trace_hooks():
    import types

    import antenv

    if not hasattr(antenv, "axon_hooks"):
        hooks_mod = types.ModuleType("antenv.axon_hooks")
        _hook = [None]
        hooks_mod.set_axon_ntff_profile_hook = lambda h: _hook.__setitem__(0, h)
        hooks_mod.get_axon_ntff_profile_hook = lambda: _hook[0]
        sys.modules["antenv.axon_hooks"] = hooks_mod
        antenv.axon_hooks = hooks_mod
        from trn_agent_boot.trn_boot import _ntff_profile_via_ctypes

        hooks_mod.set_axon_ntff_profile_hook(
            _ntff_profile_via_ctypes("/opt/axon/libaxon_pjrt.so")
        )
    bass_utils.upload_artifacts = lambda tmpdir: f"local://{tmpdir}"


def kernel(x, Wqkv, bqkv, Wo, bo):
    global LAST_RESULT
    x = np.asarray(x, dtype=np.float32)
    Wqkv = np.asarray(Wqkv, dtype=np.float32)
    bqkv = np.asarray(bqkv, dtype=np.float32)
    Wo = np.asarray(Wo, dtype=np.float32)
    bo = np.asarray(bo, dtype=np.float32)

    nc = _get_program()

    cosT, sinT = _rope_tables()
    sinT = sinT.copy()
    sinT[: HD // 2, :] *= -1.0  # rotation sign folded into the sin table
    cos16 = cosT.astype(np.float16)
    sin16 = sinT.astype(np.float16)
    onesm = np.ones((128, 128), dtype=np.float16)
    ident = np.eye(128, dtype=np.float16)
    # maskM[j, i] = MASKVAL where key j > query i within the diagonal
    # [128 keys x 128 queries] block; zero elsewhere (cols 128..511 unused).
    maskM = np.zeros((128, TB), dtype=np.float16)
    j_idx = np.arange(128)[:, None]
    i_idx = np.arange(128)[None, :]
    maskM[:, :128] = np.where(j_idx > i_idx, np.float16(MASKVAL), np.float16(0.0))

    xT = np.ascontiguousarray(x.transpose(0, 2, 1)).astype(np.float16)

    in_maps = []
    for c in range(NC_):
        rs = slice(c * NH * HD, (c + 1) * NH * HD)
        in_maps.append(
            {
                "xT": xT,
                "wq": np.ascontiguousarray(
                    Wqkv[0 * C :][rs.start : rs.stop, :].T
                ).astype(np.float16),
                "wk": np.ascontiguousarray(
                    Wqkv[1 * C :][rs.start : rs.stop, :].T
                ).astype(np.float16),
                "wv": np.ascontiguousarray(
                    Wqkv[2 * C :][rs.start : rs.stop, :].T
                ).astype(np.float16),
                "wo": np.ascontiguousarray(Wo[:, rs].T).astype(np.float16),
                "bq": np.ascontiguousarray(bqkv[0 * C :][rs].reshape(NH, HD).T),
                "bk": np.ascontiguousarray(bqkv[1 * C :][rs].reshape(NH, HD).T),
                "cosT": cos16,
                "sinT": sin16,
                "maskM": maskM,
                "ident": ident,
                "onesm": onesm,
            }
        )

    if TRACE:
        _enable_trace_hooks()
    res = bass_utils.run_bass_kernel_spmd(
        nc, in_maps, core_ids=list(range(NC_)), trace=TRACE
    )
    LAST_RESULT = res

    out = np.zeros((B, T, C), dtype=np.float64)
    for c in range(NC_):
        out += res.results[c]["out"]
    bv = bqkv[2 * C : 3 * C]
    out += (bo + Wo @ bv)[None, None, :]
    return out.astype(np.float32)


# revision 27
# speedup vs baseline: 1.0421x; 1.0421x over previous
"""Causal self-attention (B=2, T=2048, C=2048, H=16) on 8 TRN2 NeuronCores.

Sharding: tensor-parallel over heads (2 heads per core, both batches on every
core). Each core computes q/k/v projections for its 2 heads, RoPE, causal
softmax(qk^T)v, and a partial output projection against its slice of Wo's
columns. The host sums the 8 partial projections and adds the (linear) bias
terms (bo + Wo @ bv).

v2 changes vs the f32r baseline (615us -> target ~300us):
  - All matmul operands fp16 (PSUM accumulation stays f32). The correctness
    budget is huge (baseline rel err 3.4e-4 vs the 2e-2 gate); fp16 halves
    DMA traffic, doubles DVE throughput, and halves LDWEIGHTS time.
  - Causal mask folded into the score accumulation: a [128,W] matmul writes
    -60000 into the masked triangle (identity lhsT x const mask rhs,
    start=True) and the k.q matmul accumulates on top. exp then underflows
    to exactly 0. This removes the DVE mask multiply from the
    scores->exp->PV critical path.
  - Global emission scheduler: the PE executes its queue in order, so the
    Python emission order IS the schedule. Attention streams (latency-bound:
    s -> exp on ACT -> y) are round-robined two at a time, and every slot is
    padded with "bulk" matmuls (qkv projections of the *next* batch, output
    projections of finished blocks). The PE therefore never idles, which
    also keeps it at the 2.4 GHz p-state (it drops to 1.2 GHz whenever the
    pipeline has gaps - that is where the baseline lost most of its time).
  - Output projection tiles are DMA'd straight out of PSUM (f32) on the
    sync queue; no PSUM->SBUF copy.

Layout (unchanged): x pre-transposed on host to xT[b] [C,T]; q,k produced
transposed [head_dim, T]; scores S_T = k_tile.T @ qT [keys, queries] so the
softmax reduction runs along partitions via a ones-matmul; y produced
transposed yT = v_tile.T @ P_T; output projection out = yT_slice.T @ wo.
"""

import sys

sys.path.insert(0, "/opt/trn_rl_repo")

from collections import deque

import numpy as np

import concourse.bacc as bacc
import concourse.mybir as mybir
import concourse.tile as tile
from concourse import bass_utils

B, T, C, H = 2, 2048, 2048, 16
HD = C // H  # 128
BASE = 10000.0
NC_ = 8  # cores
NH = H // NC_  # heads per core = 2
TB = 512  # T block
NTB = T // TB  # 4
CK = C // 128  # 16 contraction chunks
SCALE = 1.0 / float(np.sqrt(np.float32(HD)))
MASKVAL = -60000.0

f32 = mybir.dt.float32
f16 = mybir.dt.float16
AF = mybir.ActivationFunctionType
OP = mybir.AluOpType

TRACE = False
LAST_RESULT = None
LDW_OPT = False

_orig_run_command = bass_utils.run_command


def _patched_run_command(cmd, **kw):
    if LDW_OPT and isinstance(cmd, list):
        cmd = [
            ("--enable-ldw-opt=true" if c == "--enable-ldw-opt=false" else c)
            for c in cmd
        ]
    return _orig_run_command(cmd, **kw)


bass_utils.run_command = _patched_run_command

_STATE = {}


def _rope_tables():
    """cos/sin tables [HD, T] mirroring reference._rope_tables (f32 chain)."""
    try:
        import jax
        import jax.numpy as jnp

        cpu = jax.devices("cpu")[0]
        with jax.default_device(cpu):
            p = jnp.arange(HD // 2, dtype=jnp.float32)
            theta = jnp.power(BASE, -(2.0**p) / HD)
            pos = jnp.arange(1, T + 1, dtype=jnp.float32)[:, None]
            c = pos * theta
            ang = jnp.concatenate([c, c], axis=-1)  # [T, HD]
            cos = np.asarray(jnp.cos(ang)).T  # [HD, T]
            sin = np.asarray(jnp.sin(ang)).T
        return np.ascontiguousarray(cos), np.ascontiguousarray(sin)
    except Exception:
        p = np.arange(HD // 2, dtype=np.float32)
        theta = np.power(np.float32(BASE), (-(2.0**p) / HD).astype(np.float32))
        pos = np.arange(1, T + 1, dtype=np.float32)[:, None]
        c = (pos * theta).astype(np.float32)
        ang = np.concatenate([c, c], axis=-1)
        return (
            np.ascontiguousarray(np.cos(ang).T.astype(np.float32)),
            np.ascontiguousarray(np.sin(ang).T.astype(np.float32)),
        )


def _build_program():
    nc = bacc.Bacc("TRN2", target_bir_lowering=False, debug=False, num_devices=NC_)

    d_xT = nc.dram_tensor("xT", (B, C, T), f16, kind="ExternalInput")
    d_wq = nc.dram_tensor("wq", (C, NH * HD), f16, kind="ExternalInput")
    d_wk = nc.dram_tensor("wk", (C, NH * HD), f16, kind="ExternalInput")
    d_wv = nc.dram_tensor("wv", (C, NH * HD), f16, kind="ExternalInput")
    d_wo = nc.dram_tensor("wo", (NH * HD, C), f16, kind="ExternalInput")
    d_bq = nc.dram_tensor("bq", (HD, NH), f32, kind="ExternalInput")
    d_bk = nc.dram_tensor("bk", (HD, NH), f32, kind="ExternalInput")
    d_cos = nc.dram_tensor("cosT", (HD, T), f16, kind="ExternalInput")
    d_sin = nc.dram_tensor("sinT", (HD, T), f16, kind="ExternalInput")
    d_maskM = nc.dram_tensor("maskM", (128, TB), f16, kind="ExternalInput")
    d_id = nc.dram_tensor("ident", (128, 128), f16, kind="ExternalInput")
    d_ones = nc.dram_tensor("onesm", (128, 128), f16, kind="ExternalInput")
    d_out = nc.dram_tensor("out", (B, T, C), f16, kind="ExternalOutput")

    with tile.TileContext(nc) as tc:
        with (
            tc.tile_pool(name="w", bufs=1) as wp,
            tc.tile_pool(name="xp", bufs=1) as xp,
            tc.tile_pool(name="kv", bufs=1) as kvp,
            tc.tile_pool(name="wk", bufs=1) as wkp,
            tc.tile_pool(name="ps", bufs=1, space="PSUM") as ps,
        ):
            # --- resident weights/constants ---
            # tiny consts first so PE warmup can start within ~0.5us
            maskM_t = wp.tile([128, TB], f16, name="maskM_t")
            nc.gpsimd.dma_start(maskM_t[:], d_maskM.ap()[:])
            id_t = wp.tile([128, 128], f16, name="id_t")
            nc.gpsimd.dma_start(id_t[:], d_id.ap()[:])
            ones_t = wp.tile([128, 128], f16, name="ones_t")
            nc.gpsimd.dma_start(ones_t[:], d_ones.ap()[:])
            # PE warmup: matmuls on the const tiles ramp the tensor-engine
            # p-state while the initial weight/x DMAs are still in flight.
            warm_ps = ps.tile([128, TB], f32, tag="fill", bufs=3, name="warm_ps")
            for _ in range(20):
                nc.tensor.matmul(
                    warm_ps[:], ones_t[:], maskM_t[:], skip_group_check=True
                )
            wq_t = wp.tile([128, CK, NH * HD], f16, name="wq_t")
            wk_t = wp.tile([128, CK, NH * HD], f16, name="wk_t")
            wv_t = wp.tile([128, CK, NH * HD], f16, name="wv_t")
            wq_src = d_wq.ap().rearrange("(k p) n -> p k n", p=128)
            wk_src = d_wk.ap().rearrange("(k p) n -> p k n", p=128)
            # all weight quads on the (steady) gpsimd queue; scalar only
            # carries the rope-swap DMAs, sync only carries x
            for qd in range(4):
                qs = slice(qd * 4, (qd + 1) * 4)
                nc.gpsimd.dma_start(wq_t[:, qs, :], wq_src[:, qs, :])
                nc.gpsimd.dma_start(wk_t[:, qs, :], wk_src[:, qs, :])
            bq_t = wp.tile([128, NH], f32, name="bq_t")
            nc.gpsimd.dma_start(bq_t[:], d_bq.ap()[:])
            bk_t = wp.tile([128, NH], f32, name="bk_t")
            nc.gpsimd.dma_start(bk_t[:], d_bk.ap()[:])
            cos_t = wp.tile([128, T], f16, name="cos_t")
            nc.gpsimd.dma_start(cos_t[:], d_cos.ap()[:])
            sin_t = wp.tile([128, T], f16, name="sin_t")
            nc.gpsimd.dma_start(sin_t[:], d_sin.ap()[:])
            # deferred weights (needed later than wq/wk)
            wv_src = d_wv.ap().rearrange("(k p) n -> p k n", p=128)
            for qd in range(4):
                qs = slice(qd * 4, (qd + 1) * 4)
                nc.gpsimd.dma_start(wv_t[:, qs, :], wv_src[:, qs, :])
            wo_t = wp.tile([128, NH, C], f16, name="wo_t")
            for h in range(NH):
                nc.gpsimd.dma_start(
                    wo_t[:, h, :], d_wo.ap()[h * 128 : (h + 1) * 128, :]
                )

            # --- python-side bookkeeping ---
            xchunks = {}
            qTt = {}
            kTt = {}
            vtt = {}
            yTs = {}
            p1_done = {0: 0, 1: 0}
            done_h = {}
            bulk = deque()
            counters = {"attn": 0, "bulk": 0}

            def xload(b, tb):
                tiles = []
                src = d_xT.ap()[b].rearrange("(k p) t -> p k t", p=128)
                for qd in range(4):
                    xt = xp.tile(
                        [128, 4, TB], f16, tag="x", bufs=10, name=f"x{b}_{tb}_{qd}"
                    )
                    nc.sync.dma_start(
                        xt[:],
                        src[:, qd * 4 : (qd + 1) * 4, tb * TB : (tb + 1) * TB],
                    )
                    tiles.append(xt)
                xchunks[(b, tb)] = tiles

            def qk_group(b, tb, h, proj):
                w_t, b_t = (wq_t, bq_t) if proj == 0 else (wk_t, bk_t)
                tbs = slice(tb * TB, (tb + 1) * TB)
                hsl = slice(h * HD, (h + 1) * HD)
                xs = xchunks[(b, tb)]
                prj = ps.tile([128, TB], f32, tag="fill", bufs=3, name="prj")
                for kc in range(CK):
                    nc.tensor.matmul(
                        prj[:],
                        w_t[:, kc, hsl],
                        xs[kc // 4][:, kc % 4, :],
                        start=(kc == 0),
                        stop=(kc == CK - 1),
                        skip_group_check=True,
                    )
                    yield
                qb = wkp.tile([128, TB], f16, tag="qb", bufs=4, name="qb")
                nc.scalar.activation(qb[:], prj[:], AF.Identity, bias=b_t[:, h : h + 1])
                rotb = wkp.tile([128, TB], f16, tag="qb", bufs=4, name="rotb")
                nc.scalar.dma_start(rotb[0:64, :], qb[64:128, :])
                nc.scalar.dma_start(rotb[64:128, :], qb[0:64, :])
                t1 = wkp.tile([128, TB], f16, tag="rt", bufs=4, name="t1")
                nc.vector.tensor_tensor(t1[:], qb[:], cos_t[:, tbs], OP.mult)
                t2 = wkp.tile([128, TB], f16, tag="rt", bufs=4, name="t2")
                nc.vector.tensor_tensor(t2[:], rotb[:], sin_t[:, tbs], OP.mult)
                dest = (qTt if proj == 0 else kTt)[(b, h)]
                nc.vector.tensor_tensor(dest[:, tbs], t1[:], t2[:], OP.add)

            def v_group(b, tb, tt):
                xs = xchunks[(b, tb)]
                vps = ps.tile([128, TB], f32, tag="fill", bufs=3, name="vps")
                for kc in range(CK):
                    nc.tensor.matmul(
                        vps[:, : NH * HD],
                        xs[kc // 4][:, kc % 4, tt * 128 : (tt + 1) * 128],
                        wv_t[:, kc, :],
                        start=(kc == 0),
                        stop=(kc == CK - 1),
                        skip_group_check=True,
                    )
                    yield
                if tt % 2 == 0:
                    nc.scalar.activation(
                        vtt[b][:, tb * 4 + tt, :], vps[:, : NH * HD], AF.Identity
                    )
                else:
                    nc.vector.tensor_copy(vtt[b][:, tb * 4 + tt, :], vps[:, : NH * HD])

            def p1_gen(b):
                for h in range(NH):
                    qTt[(b, h)] = kvp.tile(
                        [128, T], f16, tag=f"q{h}", bufs=2, name=f"qT{b}_{h}"
                    )
                    kTt[(b, h)] = kvp.tile(
                        [128, T], f16, tag=f"k{h}", bufs=2, name=f"kT{b}_{h}"
                    )
                vtt[b] = kvp.tile(
                    [128, CK, NH * HD], f16, tag="v", bufs=2, name=f"v{b}"
                )
                for tb in range(NTB):
                    xload(b, tb)
                    for proj in range(2):
                        for h in range(NH):
                            yield from qk_group(b, tb, h, proj)
                    for tt in range(4):
                        yield from v_group(b, tb, tt)
                    p1_done[b] = tb + 1

            def out_group(b, tb, tt, ncc):
                o_ps = ps.tile([128, TB], f32, tag="fill", bufs=3, name="o_ps")
                for h in range(NH):
                    nc.tensor.matmul(
                        o_ps[:],
                        yTs[(b, tb, h)][:, tt * 128 : (tt + 1) * 128],
                        wo_t[:, h, ncc * TB : (ncc + 1) * TB],
                        start=(h == 0),
                        stop=(h == NH - 1),
                        skip_group_check=True,
                    )
                    yield
                r0 = tb * TB + tt * 128
                ot = wkp.tile([128, TB], f16, tag="ot", bufs=6, name="ot")
                if ncc % 2 == 0:
                    nc.scalar.activation(ot[:], o_ps[:], AF.Identity)
                else:
                    nc.vector.tensor_copy(ot[:], o_ps[:])
                dq = nc.gpsimd if ncc % 2 == 0 else nc.sync
                dq.dma_start(
                    d_out.ap()[b, r0 : r0 + 128, ncc * TB : (ncc + 1) * TB], ot[:]
                )

            def attn_stream(b, h, tb):
                nkt = 4 * tb + 4
                hsl = slice(h * HD, (h + 1) * HD)
                q = qTt[(b, h)]
                k = kTt[(b, h)]
                v = vtt[b]
                y_ps = ps.tile([128, TB], f32, tag="y", bufs=2, name="y_ps")
                dacc = wkp.tile([128, TB], f16, tag="dacc", bufs=4, name="dacc")
                pend = deque()
                LAG = 1

                def emit_y(item):
                    kt_, q0_, W_, pt_ = item
                    nc.tensor.matmul(
                        y_ps[:, q0_:],
                        v[:, kt_, hsl],
                        pt_[:, :W_],
                        start=(kt_ == 0),
                        stop=(kt_ == nkt - 1),
                        skip_group_check=True,
                    )

                for kt in range(nkt):
                    o = kt - 4 * tb
                    q0 = 128 * o if o > 0 else 0
                    W = TB - q0
                    s_ps = ps.tile([128, TB], f32, tag="s", bufs=3, name="s_ps")
                    if o >= 0:
                        nc.tensor.matmul(
                            s_ps[:, :W],
                            k[:, kt * 128 : (kt + 1) * 128],
                            q[:, tb * TB + q0 : (tb + 1) * TB],
                            start=True,
                            stop=False,
                            skip_group_check=True,
                        )
                        yield
                        nc.tensor.matmul(
                            s_ps[:, :128],
                            id_t[:],
                            maskM_t[:, :128],
                            start=False,
                            stop=True,
                            skip_group_check=True,
                        )
                        yield
                    else:
                        nc.tensor.matmul(
                            s_ps[:, :W],
                            k[:, kt * 128 : (kt + 1) * 128],
                            q[:, tb * TB + q0 : (tb + 1) * TB],
                            start=True,
                            stop=True,
                            skip_group_check=True,
                        )
                        yield
                    pt = wkp.tile([128, TB], f16, tag="p", bufs=9, name="pt")
                    nc.scalar.activation(pt[:, :W], s_ps[:, :W], AF.Exp, scale=SCALE)
                    if kt == 0:
                        nc.vector.tensor_copy(dacc[:], pt[:])
                    else:
                        nc.vector.tensor_tensor(
                            dacc[:, q0:], dacc[:, q0:], pt[:, :W], OP.add
                        )
                    pend.append((kt, q0, W, pt))
                    if len(pend) > LAG:
                        emit_y(pend.popleft())
                        yield
                while pend:
                    emit_y(pend.popleft())
                    yield
                den_ps = ps.tile([128, TB], f32, tag="s", bufs=3, name="den_ps")
                nc.tensor.matmul(den_ps[:], ones_t[:], dacc[:], skip_group_check=True)
                rden = wkp.tile([128, TB], f32, tag="rden", bufs=3, name="rden")
                nc.vector.reciprocal_approx_fast(rden[:], den_ps[:])
                yT = wkp.tile([128, TB], f16, tag="yT", bufs=6, name="yT")
                nc.vector.tensor_tensor(yT[:], y_ps[:], rden[:], OP.mult)
                yTs[(b, tb, h)] = yT
                done_h[(b, tb)] = done_h.get((b, tb), 0) + 1
                if done_h[(b, tb)] == NH:
                    for tt in range(4):
                        for ncc in range(4):
                            bulk.append(("p3", out_group(b, tb, tt, ncc)))
                            counters["bulk"] += 2

            # --- schedule ---
            streams = deque()
            for b in range(B):
                for tb in range(NTB):
                    for h in range(NH):
                        nkt = 4 * tb + 4
                        streams.append((b, h, tb))
                        counters["attn"] += 2 * nkt + 4

            bulk.append(("p1", p1_gen(0)))
            counters["bulk"] += 512
            bulk.append(("p1", p1_gen(1)))
            counters["bulk"] += 512

            active = []

            def refill():
                while len(active) < 2 and streams:
                    b, h, tb = streams[0]
                    if p1_done[b] >= tb + 1:
                        streams.popleft()
                        active.append(attn_stream(b, h, tb))
                    else:
                        break

            bulk_rr = []

            def bulk_step(n):
                # Step up to two bulk generators round-robin so independent
                # out-proj groups pipeline; never run two "p1" (projection)
                # generators concurrently (their x-chunk DMA order matters).
                emitted = 0
                while emitted < n:
                    while len(bulk_rr) < 2 and bulk:
                        if bulk[0][0] == "p1" and any(
                            k == "p1" for k, _ in bulk_rr
                        ):
                            break
                        bulk_rr.append(bulk.popleft())
                    if not bulk_rr:
                        return
                    item = bulk_rr.pop(0)
                    bulk_rr.append(item)
                    try:
                        next(item[1])
                        emitted += 1
                        counters["bulk"] -= 1
                    except StopIteration:
                        bulk_rr.remove(item)

            while True:
                refill()
                if not active and not bulk and not bulk_rr and not streams:
                    break
                if active:
                    for st in list(active):
                        try:
                            next(st)
                            counters["attn"] -= 1
                        except StopIteration:
                            active.remove(st)
                    r = max(
                        1,
                        min(4, round(counters["bulk"] / max(counters["attn"], 1))),
                    )
                    bulk_step(r)
                else:
                    bulk_step(4)

    nc.compile()
    return nc


def _get_program():
    if "nc" not in _STATE:
        _STATE["nc"] = _build_program()
    return _STATE["nc"]


def _enable_trace_hooks():
    import types

    import antenv

    if not hasattr(antenv, "axon_hooks"):
        hooks_mod = types.ModuleType("antenv.axon_hooks")
        _hook = [None]
        hooks_mod.set_axon_ntff_profile_hook = lambda h: _hook.__setitem__(0, h)
        hooks_mod.get_axon_ntff_profile_hook = lambda: _hook[0]
        sys.modules["antenv.axon_hooks"] = hooks_mod
        antenv.axon_hooks = hooks_mod
        from trn_agent_boot.trn_boot import _ntff_profile_via_ctypes

        hooks_mod.set_axon_ntff_profile_hook(
            _ntff_profile_via_ctypes("/opt/axon/libaxon_pjrt.so")
        )
    bass_utils.upload_artifacts = lambda tmpdir: f"local://{tmpdir}"


def kernel(x, Wqkv, bqkv, Wo, bo):
    global LAST_RESULT
    x = np.asarray(x, dtype=np.float32)
    Wqkv = np.asarray(Wqkv, dtype=np.float32)
    bqkv = np.asarray(bqkv, dtype=np.float32)
    Wo = np.asarray(Wo, dtype=np.float32)
    bo = np.asarray(bo, dtype=np.float32)

    nc = _get_program()

    cosT, sinT = _rope_tables()
    sinT = sinT.copy()
    sinT[: HD // 2, :] *= -1.0  # rotation sign folded into the sin table
    cos16 = cosT.astype(np.float16)
    sin16 = sinT.astype(np.float16)
    onesm = np.ones((128, 128), dtype=np.float16)
    ident = np.eye(128, dtype=np.float16)
    # maskM[j, i] = MASKVAL where key j > query i within the diagonal
    # [128 keys x 128 queries] block; zero elsewhere (cols 128..511 unused).
    maskM = np.zeros((128, TB), dtype=np.float16)
    j_idx = np.arange(128)[:, None]
    i_idx = np.arange(128)[None, :]
    maskM[:, :128] = np.where(j_idx > i_idx, np.float16(MASKVAL), np.float16(0.0))

    xT = np.ascontiguousarray(x.transpose(0, 2, 1)).astype(np.float16)

    in_maps = []
    for c in range(NC_):
        rs = slice(c * NH * HD, (c + 1) * NH * HD)
        in_maps.append(
            {
                "xT": xT,
                "wq": np.ascontiguousarray(
                    Wqkv[0 * C :][rs.start : rs.stop, :].T
                ).astype(np.float16),
                "wk": np.ascontiguousarray(
                    Wqkv[1 * C :][rs.start : rs.stop, :].T
                ).astype(np.float16),
                "wv": np.ascontiguousarray(
                    Wqkv[2 * C :][rs.start : rs.stop, :].T
                ).astype(np.float16),
                "wo": np.ascontiguousarray(Wo[:, rs].T).astype(np.float16),
                "bq": np.ascontiguousarray(bqkv[0 * C :][rs].reshape(NH, HD).T),
                "bk": np.ascontiguousarray(bqkv[1 * C :][rs].reshape(NH, HD).T),
                "cosT": cos16,
                "sinT": sin16,
                "maskM": maskM,
                "ident": ident,
                "onesm": onesm,
            }
        )

    if TRACE:
        _enable_trace_hooks()
    res = bass_utils.run_bass_kernel_spmd(
        nc, in_maps, core_ids=list(range(NC_)), trace=TRACE
    )
    LAST_RESULT = res

    out = np.zeros((B, T, C), dtype=np.float64)
    for c in range(NC_):
        out += res.results[c]["out"]
    bv = bqkv[2 * C : 3 * C]
    out += (bo + Wo @ bv)[None, None, :]
    return out.astype(np.float32)
